# revision 5
# baseline (speedup 1.0000x reference)
"""MLA absorbed-QKVO attention kernel for Trainium2 (8 NeuronCores).

Sharding: heads (H=16) tensor-parallel across 8 cores, 2 heads/core.
Host slices w_qb rows / w_o cols per core; each core computes a partial
output (its 2 heads through w_o) and the host sums the 8 partials.

On-device pipeline per core (all GEMM operands bf16, fp32 PSUM accum):
  1. cast f32 inputs -> bf16 DRAM scratch
  2. per s-block of 512 tokens:
     hidT (DMA-transpose) -> q_a row-major -> q_aT (DMA-T) -> q row-major
     kv row-major (token-major value stays resident per batch)
     keyT (d-major, DMA-T) + RoPE, queryT (d-major, DMA-T) + RoPE
     scores = qT.T @ keyT (causal-masked via affine_select, softmax fp32)
     P -> PE-transpose -> PT;  attnoutT = value.T @ PT
     out_partial = attnoutT.T @ w_oT  (streamed)  -> DRAM
"""

import sys

import numpy as np

if "/opt/trn_rl_repo" not in sys.path:
    sys.path.insert(0, "/opt/trn_rl_repo")

B, S, HID = 2, 2048, 2048
H = 16
QK_ROPE = 64
KVR = 512
QLR = 1536
KVD = 640          # w_kv output dim: 64 rope + 64 vrope + 512 nope
DHEAD = 576
N_CORES = 8
HPC = H // N_CORES  # heads per core
OC = HPC * DHEAD    # 1152
SCALE = 1.0 / float(np.sqrt(128.0))

P = 128
SBLK = 512
KT = 512
NEG = -1e30


def build_nc(b_count=B, s_len=S):
    import concourse.bass as bass  # noqa: F401
    import concourse.mybir as mybir
    import concourse.tile as tile
    from concourse import bacc
    from concourse.masks import make_identity

    fp32 = mybir.dt.float32
    bf16 = mybir.dt.bfloat16
    Exp = mybir.ActivationFunctionType.Exp
    AX = mybir.AxisListType.X
    MAX = mybir.AluOpType.max
    GE = mybir.AluOpType.is_ge

    NB = s_len // SBLK
    NSC = SBLK // P            # 4 q-subtiles / block
    NKC = HID // P             # 16 hid chunks
    NQLC = QLR // P            # 12 q-lora chunks
    NTOK = s_len // P          # token chunks per batch
    R = b_count * s_len

    nc = bacc.Bacc(None, target_bir_lowering=False)

    hidden = nc.dram_tensor("hidden", [R, HID], fp32, kind="ExternalInput")
    ropeT_in = nc.dram_tensor("ropeT", [P, s_len], fp32, kind="ExternalInput")
    w_qa = nc.dram_tensor("w_qa", [QLR, HID], fp32, kind="ExternalInput")
    w_qb = nc.dram_tensor("w_qb_h", [OC, QLR], fp32, kind="ExternalInput")
    w_kv = nc.dram_tensor("w_kv", [KVD, HID], fp32, kind="ExternalInput")
    w_o = nc.dram_tensor("w_o_h", [HID, OC], fp32, kind="ExternalInput")
    out_d = nc.dram_tensor("out_part", [R, HID], fp32, kind="ExternalOutput")

    # value-dim chunking for AV / attnoutT packed over both heads (9x128 rows)
    # entries: (chunk_idx, row_off, kv_col_off, width)
    AV_CHUNKS = [
        [(0, 0, 64, 128), (1, 0, 192, 128), (2, 0, 320, 128),
         (3, 0, 448, 128), (4, 0, 576, 64)],
        [(4, 64, 64, 64), (5, 0, 128, 128), (6, 0, 256, 128),
         (7, 0, 384, 128), (8, 0, 512, 128)],
    ]
    # q/k d-chunking per head: (chunk_slot, d_off_in_head, width); slot 0 = rope
    QK_CHUNKS = [(0, 0, 64), (1, 64, 128), (2, 192, 128),
                 (3, 320, 128), (4, 448, 128)]

    with tile.TileContext(nc) as tc:
        with (
            tc.tile_pool(name="dram", bufs=1, space="DRAM") as dram,
            tc.tile_pool(name="singles", bufs=1) as singles,
            tc.tile_pool(name="cast", bufs=2) as cast,
            tc.tile_pool(name="strm", bufs=1) as strm,
            tc.tile_pool(name="work", bufs=1) as work,
            tc.tile_pool(name="stats", bufs=4) as stats,
            tc.tile_pool(name="rtmp", bufs=2) as rtmp,
            tc.tile_pool(name="psA", bufs=4, space="PSUM") as psA,
            tc.tile_pool(name="psS", bufs=2, space="PSUM") as psS,
            tc.tile_pool(name="psV", bufs=2, space="PSUM") as psV,
        ):
            # ---------------- DRAM bf16 scratch ----------------
            hbf = [dram.tile([SBLK, HID], bf16, tag=f"hbf{i}", name=f"hbf{i}")
                   for i in range(R // SBLK)]
            w_qa_bf = dram.tile([QLR, HID], bf16, tag="wqabf", name="wqabf")
            w_qb_bf = dram.tile([OC, QLR], bf16, tag="wqbbf", name="wqbbf")
            w_kv_bf = dram.tile([KVD, HID], bf16, tag="wkvbf", name="wkvbf")
            w_o_bf = dram.tile([HID, OC], bf16, tag="wobf", name="wobf")

            cast_engines = [
                lambda o, i: nc.vector.tensor_copy(out=o, in_=i),
                lambda o, i: nc.scalar.copy(out=o, in_=i),
                lambda o, i: nc.gpsimd.tensor_copy(out=o, in_=i),
            ]
            cast_i = [0]

            def cast_bf16(dst, src, rows, cols):
                for r0 in range(0, rows, P):
                    for c0 in range(0, cols, 1024):
                        cw = min(1024, cols - c0)
                        ti = cast.tile([P, 1024], fp32, tag="ci", name="ci")
                        to = cast.tile([P, 1024], bf16, tag="co", name="co")
                        nc.gpsimd.dma_start(
                            out=ti[:, :cw], in_=src[r0:r0 + P, c0:c0 + cw])
                        eng = cast_engines[cast_i[0] % 3]
                        cast_i[0] += 1
                        eng(to[:, :cw], ti[:, :cw])
                        nc.gpsimd.dma_start(
                            out=dst[r0:r0 + P, c0:c0 + cw], in_=to[:, :cw])

            cast_bf16(w_qa_bf, w_qa, QLR, HID)
            cast_bf16(w_kv_bf, w_kv, KVD, HID)
            cast_bf16(w_qb_bf, w_qb, OC, QLR)
            cast_bf16(w_o_bf, w_o, HID, OC)
            for i in range(R // SBLK):
                cast_bf16(hbf[i], hidden[i * SBLK:(i + 1) * SBLK, :], SBLK, HID)

            # ---------------- constants ----------------
            ropeT = singles.tile([P, s_len], fp32, name="ropeTs")
            nc.sync.dma_start(out=ropeT[:, :], in_=ropeT_in[:, :])
            ident = singles.tile([P, P], fp32, name="identf")
            make_identity(nc, ident[:, :])
            ident_b = singles.tile([P, P], bf16, name="identb")
            make_identity(nc, ident_b[:, :])

            def rope_apply(dst_hi, dst_lo, src, cols):
                """dst[0:32] = x1*cos1 - x2*sin1 ; dst[32:64] = x2*cos2 + x1*sin2.
                src: fp32 AP [64, w] (d-major);  cols: global token slice."""
                w = src.shape[-1]
                a = rtmp.tile([32, P], fp32, tag="rt0", name="rt0")
                bb = rtmp.tile([32, P], fp32, tag="rt1", name="rt1")
                nc.vector.tensor_mul(a[:, :w], src[0:32, :], ropeT[0:32, cols])
                nc.vector.tensor_mul(bb[:, :w], src[32:64, :], ropeT[64:96, cols])
                nc.vector.tensor_sub(dst_hi, a[:, :w], bb[:, :w])
                nc.vector.tensor_mul(a[:, :w], src[32:64, :], ropeT[32:64, cols])
                nc.vector.tensor_mul(bb[:, :w], src[0:32, :], ropeT[96:128, cols])
                nc.vector.tensor_add(dst_lo, a[:, :w], bb[:, :w])

            # ---------------- main loop ----------------
            for b in range(b_count):
                kv_r = work.tile([P, NTOK, KVD], bf16, tag="kv_r", name="kv_r")
                keyT = work.tile([P, 5, s_len], bf16, tag="keyT", name="keyT")

                for blk in range(NB):
                    rows0 = b * s_len + blk * SBLK
                    nblk = b * NB + blk

                    hidT = work.tile([P, NKC, SBLK], bf16, tag="hidT",
                                     name="hidT")
                    for kc in range(NKC):
                        nc.sync.dma_start_transpose(
                            out=hidT[:, kc, :],
                            in_=hbf[nblk][:, kc * P:(kc + 1) * P])

                    # ---- q_a = hidden @ w_qa.T (row-major) ----
                    q_a_sb = work.tile([P, NSC, QLR], bf16, tag="q_a_sb",
                                       name="q_a_sb")
                    for ct in range(QLR // KT):
                        pss = [psA.tile([P, KT], fp32, tag="psA", name="psA")
                               for _ in range(NSC)]
                        for kc in range(NKC):
                            wt = strm.tile([P, KT], bf16, tag="w_qaT", bufs=4,
                                           name="w_qaT")
                            nc.sync.dma_start_transpose(
                                out=wt[:, :],
                                in_=w_qa_bf[ct * KT:(ct + 1) * KT,
                                            kc * P:(kc + 1) * P])
                            for sc in range(NSC):
                                nc.tensor.matmul(
                                    pss[sc][:, :],
                                    hidT[:, kc, sc * P:(sc + 1) * P],
                                    wt[:, :],
                                    start=(kc == 0), stop=(kc == NKC - 1))
                        for sc in range(NSC):
                            nc.scalar.copy(
                                out=q_a_sb[:, sc, ct * KT:(ct + 1) * KT],
                                in_=pss[sc][:, :])

                    # ---- kv = hidden @ w_kv.T (row-major, token-major value) ----
                    for noff, nw in [(0, 512), (512, 128)]:
                        pss = [psA.tile([P, KT], fp32, tag="psA", name="psA")
                               for _ in range(NSC)]
                        for kc in range(NKC):
                            wt = strm.tile([P, KVD], bf16, tag="w_kvT", bufs=3,
                                           name="w_kvT")
                            nc.sync.dma_start_transpose(
                                out=wt[:, :],
                                in_=w_kv_bf[:, kc * P:(kc + 1) * P])
                            for sc in range(NSC):
                                nc.tensor.matmul(
                                    pss[sc][:, :nw],
                                    hidT[:, kc, sc * P:(sc + 1) * P],
                                    wt[:, noff:noff + nw],
                                    start=(kc == 0), stop=(kc == NKC - 1))
                        for sc in range(NSC):
                            nc.vector.tensor_copy(
                                out=kv_r[:, blk * NSC + sc, noff:noff + nw],
                                in_=pss[sc][:, :nw])

                    # ---- keyT assembly (+RoPE on k_rope) ----
                    for sc in range(NSC):
                        tkc = blk * NSC + sc
                        cols = slice(blk * SBLK + sc * P,
                                     blk * SBLK + (sc + 1) * P)
                        for j in range(4):
                            nc.sync.dma_start_transpose(
                                out=keyT[:, 1 + j, cols],
                                in_=kv_r[:, tkc, 128 + j * P:128 + (j + 1) * P])
                        t64 = rtmp.tile([P, 64], fp32, tag="t64", name="t64")
                        nc.vector.tensor_copy(out=t64[:, :],
                                              in_=kv_r[:, tkc, 0:64])
                        pt = psS.tile([P, KT], fp32, tag="psS", name="psS")
                        nc.tensor.transpose(pt[0:64, 0:P], t64[:, :],
                                            ident[:, :])
                        rope_apply(keyT[0:32, 0, cols], keyT[32:64, 0, cols],
                                   pt[0:64, 0:P], cols)

                    # ---- q_aT (SBUF DMA-transpose) ----
                    q_aT = work.tile([P, NQLC, SBLK], bf16, tag="q_aT",
                                     name="q_aT")
                    for sc in range(NSC):
                        for qlc in range(NQLC):
                            nc.sync.dma_start_transpose(
                                out=q_aT[:, qlc, sc * P:(sc + 1) * P],
                                in_=q_a_sb[:, sc, qlc * P:(qlc + 1) * P])

                    # ---- q = q_a @ w_qb.T (row-major) ----
                    q_sb = work.tile([P, NSC, OC], bf16, tag="q_sb",
                                     name="q_sb")
                    for ooff, otw in [(0, 512), (512, 512), (1024, 128)]:
                        pss = [psA.tile([P, KT], fp32, tag="psA", name="psA")
                               for _ in range(NSC)]
                        for qlc in range(NQLC):
                            wt = strm.tile([P, KT], bf16, tag="w_qbT", bufs=4,
                                           name="w_qbT")
                            nc.sync.dma_start_transpose(
                                out=wt[:, :otw],
                                in_=w_qb_bf[ooff:ooff + otw,
                                            qlc * P:(qlc + 1) * P])
                            for sc in range(NSC):
                                nc.tensor.matmul(
                                    pss[sc][:, :otw],
                                    q_aT[:, qlc, sc * P:(sc + 1) * P],
                                    wt[:, :otw],
                                    start=(qlc == 0), stop=(qlc == NQLC - 1))
                        for sc in range(NSC):
                            nc.scalar.copy(
                                out=q_sb[:, sc, ooff:ooff + otw],
                                in_=pss[sc][:, :otw])

                    # ---- queryT (d-major per head) + RoPE ----
                    queryT = work.tile([P, 2 * 5, SBLK], bf16, tag="queryT",
                                       name="queryT")
                    for hh in range(HPC):
                        for sc in range(NSC):
                            lcols = slice(sc * P, (sc + 1) * P)
                            gcols = slice(blk * SBLK + sc * P,
                                          blk * SBLK + (sc + 1) * P)
                            for slot, doff, dw in QK_CHUNKS[1:]:
                                so = hh * DHEAD + doff
                                nc.sync.dma_start_transpose(
                                    out=queryT[:, hh * 5 + slot, lcols],
                                    in_=q_sb[:, sc, so:so + dw])
                            t64 = rtmp.tile([P, 64], fp32, tag="t64",
                                            name="t64")
                            nc.vector.tensor_copy(
                                out=t64[:, :],
                                in_=q_sb[:, sc, hh * DHEAD:hh * DHEAD + 64])
                            pt = psS.tile([P, KT], fp32, tag="psS", name="psS")
                            nc.tensor.transpose(pt[0:64, 0:P], t64[:, :],
                                                ident[:, :])
                            rope_apply(queryT[0:32, hh * 5, lcols],
                                       queryT[32:64, hh * 5, lcols],
                                       pt[0:64, 0:P], gcols)

                    # ---- attention per head ----
                    attnoutT = work.tile([P, 9, SBLK], bf16, tag="attnoutT",
                                         name="attnoutT")
                    for hh in range(HPC):
                        PT = work.tile([P, NTOK, SBLK], bf16, tag="PT",
                                       name="PT")
                        for sc in range(NSC):
                            klen = (blk + 1) * KT
                            srow = work.tile([P, s_len], fp32, tag="srow",
                                             bufs=2, name="srow")
                            for kt in range(blk + 1):
                                ps_s = psS.tile([P, KT], fp32, tag="psS",
                                                name="psS")
                                for slot, doff, dw in QK_CHUNKS:
                                    nc.tensor.matmul(
                                        ps_s[:, :],
                                        queryT[0:dw, hh * 5 + slot,
                                               sc * P:(sc + 1) * P],
                                        keyT[0:dw, slot,
                                             kt * KT:(kt + 1) * KT],
                                        start=(slot == 0), stop=(slot == 4))
                                nc.vector.tensor_copy(
                                    out=srow[:, kt * KT:(kt + 1) * KT],
                                    in_=ps_s[:, :])
                            # causal mask on the diagonal k-tile
                            nc.gpsimd.affine_select(
                                out=srow[:, blk * KT:(blk + 1) * KT],
                                in_=srow[:, blk * KT:(blk + 1) * KT],
                                compare_op=GE, fill=NEG, base=sc * P,
                                pattern=[[-1, KT]], channel_multiplier=1)
                            mx = stats.tile([P, 1], fp32, tag="mx", name="mx")
                            nc.vector.tensor_reduce(
                                mx[:, :], srow[:, 0:klen], axis=AX, op=MAX)
                            negb = stats.tile([P, 1], fp32, tag="negb",
                                              name="negb")
                            nc.vector.tensor_scalar_mul(
                                negb[:, :], mx[:, :], -SCALE)
                            ssum = stats.tile([P, 1], fp32, tag="ssum",
                                              name="ssum")
                            nc.scalar.activation(
                                srow[:, 0:klen], srow[:, 0:klen], Exp,
                                bias=negb[:, :], scale=SCALE,
                                accum_out=ssum[:, :])
                            rec = stats.tile([P, 1], fp32, tag="rec",
                                             name="rec")
                            nc.vector.reciprocal(rec[:, :], ssum[:, :])
                            nc.scalar.mul(srow[:, 0:klen], srow[:, 0:klen],
                                          rec[:, :])
                            # transpose P -> PT
                            for g in range(blk + 1):
                                ps_t = psS.tile([P, KT], fp32, tag="psS",
                                                name="psS")
                                for j in range(4):
                                    nc.tensor.transpose(
                                        ps_t[:, j * P:(j + 1) * P],
                                        srow[:, (g * 4 + j) * P:
                                             (g * 4 + j + 1) * P],
                                        ident[:, :])
                                nc.vector.tensor_copy(
                                    out=PT[:, g * 4:(g + 1) * 4,
                                           sc * P:(sc + 1) * P],
                                    in_=ps_t.rearrange("p (a c) -> p a c", a=4))
                        # ---- attnoutT = value.T @ P.T ----
                        ntok_b = (blk + 1) * NSC
                        for ci, ro, vo, vw in AV_CHUNKS[hh]:
                            ps_v = psV.tile([P, SBLK], fp32, tag="psV",
                                            name="psV")
                            for tkc in range(ntok_b):
                                nc.tensor.matmul(
                                    ps_v[0:vw, :],
                                    kv_r[:, tkc, vo:vo + vw],
                                    PT[:, tkc, :],
                                    start=(tkc == 0), stop=(tkc == ntok_b - 1))
                            nc.scalar.copy(
                                out=attnoutT[ro:ro + vw, ci, :],
                                in_=ps_v[0:vw, :])

                    # ---- out_partial = attnout @ w_o.T ----
                    for ct in range(HID // KT):
                        wots = []
                        for hd in range(9):
                            wt = strm.tile([P, KT], bf16, tag="w_oT", bufs=11,
                                           name="w_oT")
                            nc.sync.dma_start_transpose(
                                out=wt[:, :],
                                in_=w_o_bf[ct * KT:(ct + 1) * KT,
                                           hd * P:(hd + 1) * P])
                            wots.append(wt)
                        for sc in range(NSC):
                            ps_o = psA.tile([P, KT], fp32, tag="psA",
                                            name="psA")
                            for hd in range(9):
                                nc.tensor.matmul(
                                    ps_o[:, :],
                                    attnoutT[:, hd, sc * P:(sc + 1) * P],
                                    wots[hd][:, :],
                                    start=(hd == 0), stop=(hd == 8))
                            ot = work.tile([P, KT], fp32, tag="out_sb",
                                           bufs=2, name="out_sb")
                            nc.vector.tensor_copy(out=ot[:, :], in_=ps_o[:, :])
                            nc.gpsimd.dma_start(
                                out=out_d[rows0 + sc * P:rows0 + (sc + 1) * P,
                                          ct * KT:(ct + 1) * KT],
                                in_=ot[:, :])

    nc.compile()
    return nc


def make_in_maps(inputs, b_count=B, s_len=S):
    hidden = np.ascontiguousarray(
        np.asarray(inputs["hidden_states"], dtype=np.float32).reshape(
            b_count * s_len, HID))
    cos = np.asarray(inputs["cos"], dtype=np.float32)
    sin = np.asarray(inputs["sin"], dtype=np.float32)
    ropeT = np.ascontiguousarray(
        np.concatenate([cos[0].T, sin[0].T], axis=0))  # [128, s_len]
    w_qa = np.ascontiguousarray(np.asarray(inputs["w_qa"], np.float32))
    w_qb = np.asarray(inputs["w_qb"], dtype=np.float32)
    w_kv = np.ascontiguousarray(np.asarray(inputs["w_kv"], np.float32))
    w_o = np.asarray(inputs["w_o"], dtype=np.float32)
    in_maps = []
    for c in range(N_CORES):
        in_maps.append({
            "hidden": hidden,
            "ropeT": ropeT,
            "w_qa": w_qa,
            "w_qb_h": np.ascontiguousarray(w_qb[c * OC:(c + 1) * OC, :]),
            "w_kv": w_kv,
            "w_o_h": np.ascontiguousarray(w_o[:, c * OC:(c + 1) * OC]),
        })
    return in_maps


_NC_CACHE = {}


def run_on_hw(inputs, trace=False):
    from concourse.bass_utils import run_bass_kernel_spmd

    key = "full"
    if key not in _NC_CACHE:
        _NC_CACHE[key] = build_nc()
    nc = _NC_CACHE[key]
    in_maps = make_in_maps(inputs)
    res = run_bass_kernel_spmd(nc, in_maps, core_ids=list(range(N_CORES)),
                               trace=trace)
    acc = np.zeros((B * S, HID), dtype=np.float32)
    for r in res.results:
        acc += r["out_part"]
    return acc.reshape(B, S, HID), res


def kernel(**inputs):
    out, _ = run_on_hw(inputs, trace=False)
    return out


# revision 18
# speedup vs baseline: 11.0238x; 11.0238x over previous
"""MLA absorbed-QKVO attention kernel for Trainium2 (8 NeuronCores).

Sharding: heads (H=16) tensor-parallel across 8 cores, 2 heads/core.
Host slices w_qb rows / w_o cols per core; each core computes a partial
output (its 2 heads through w_o) and the host sums the 8 partials.

v2: all transposes via PE (one-time cast+transpose pass writes transposed
bf16 weights/hidden to DRAM; per-block activation transposes via PE) —
no InstDmaTransposeAnt in steady state (HWDGE was the v1 bottleneck).
"""

import sys

import numpy as np

if "/opt/trn_rl_repo" not in sys.path:
    sys.path.insert(0, "/opt/trn_rl_repo")

B, S, HID = 2, 2048, 2048
H = 16
QK_ROPE = 64
KVR = 512
QLR = 1536
KVD = 640
DHEAD = 576
N_CORES = 8
HPC = H // N_CORES
OC = HPC * DHEAD    # 1152
SCALE = 1.0 / float(np.sqrt(128.0))

P = 128
SBLK = 512
KT = 512
NEG = -1e30


def build_nc(b_count=B, s_len=S):
    import concourse.bass as bass  # noqa: F401
    import concourse.mybir as mybir
    import concourse.tile as tile
    from concourse import bacc
    from concourse.masks import make_identity

    fp32 = mybir.dt.float32
    bf16 = mybir.dt.bfloat16
    Exp = mybir.ActivationFunctionType.Exp
    AX = mybir.AxisListType.X
    MAX = mybir.AluOpType.max
    GE = mybir.AluOpType.is_ge

    NB = s_len // SBLK
    NSC = SBLK // P
    NKC = HID // P
    NQLC = QLR // P
    NTOK = s_len // P
    R = b_count * s_len

    nc = bacc.Bacc(None, target_bir_lowering=False)

    hidden = nc.dram_tensor("hidden", [R, HID], fp32, kind="ExternalInput")
    ropeT_in = nc.dram_tensor("ropeT", [P, s_len], fp32, kind="ExternalInput")
    w_qa = nc.dram_tensor("w_qa", [QLR, HID], fp32, kind="ExternalInput")
    w_qb = nc.dram_tensor("w_qb_h", [OC, QLR], fp32, kind="ExternalInput")
    w_kv = nc.dram_tensor("w_kv", [KVD, HID], fp32, kind="ExternalInput")
    w_o = nc.dram_tensor("w_o_h", [HID, OC], fp32, kind="ExternalInput")
    out_d = nc.dram_tensor("out_part", [R, HID], fp32, kind="ExternalOutput")

    AV_CHUNKS = [
        [(0, 0, 64, 128), (1, 0, 192, 128), (2, 0, 320, 128),
         (3, 0, 448, 128), (4, 0, 576, 64)],
        [(4, 64, 64, 64), (5, 0, 128, 128), (6, 0, 256, 128),
         (7, 0, 384, 128), (8, 0, 512, 128)],
    ]
    QK_CHUNKS = [(0, 0, 64), (1, 64, 128), (2, 192, 128),
                 (3, 320, 128), (4, 448, 128)]

    with tile.TileContext(nc) as tc:
        with (
            tc.tile_pool(name="dram", bufs=1, space="DRAM") as dram,
            tc.tile_pool(name="singles", bufs=1) as singles,
            tc.tile_pool(name="cast", bufs=2) as cast,
            tc.tile_pool(name="strm", bufs=1) as strm,
            tc.tile_pool(name="work", bufs=1) as work,
            tc.tile_pool(name="stats", bufs=4) as stats,
            tc.tile_pool(name="rtmp", bufs=2) as rtmp,
            tc.tile_pool(name="psA", bufs=4, space="PSUM") as psA,
            tc.tile_pool(name="psS", bufs=2, space="PSUM") as psS,
            tc.tile_pool(name="psV", bufs=2, space="PSUM") as psV,
        ):
            # transposed bf16 scratch in DRAM
            hbfT = [dram.tile([HID, SBLK], bf16, tag=f"hbfT{i}",
                              name=f"hbfT{i}") for i in range(R // SBLK)]
            w_qaT_d = dram.tile([HID, QLR], bf16, tag="wqaT", name="wqaT")
            w_qbT_d = dram.tile([QLR, OC], bf16, tag="wqbT", name="wqbT")
            w_kvT_d = dram.tile([HID, KVD], bf16, tag="wkvT", name="wkvT")
            w_oT_d = dram.tile([OC, HID], bf16, tag="woT", name="woT")

            ident = singles.tile([P, P], fp32, name="identf")
            make_identity(nc, ident[:, :])
            ident_b = singles.tile([P, P], bf16, name="identb")
            make_identity(nc, ident_b[:, :])

            cp_eng = [lambda o, i: nc.vector.tensor_copy(out=o, in_=i),
                      lambda o, i: nc.scalar.copy(out=o, in_=i)]
            cast_i = [0]

            def cast_T(dstT, src, rows, cols):
                """dstT[c, r] <- bf16(src[r, c]) via PE transpose.

                One load + one store per [128 x 1024] source tile; the store
                covers 8 transposed row-chunks of dstT via a strided AP."""
                for r0 in range(0, rows, P):
                    for c0 in range(0, cols, 1024):
                        cw = min(1024, cols - c0)
                        ti = work.tile([P, 1024], fp32, tag="srow", bufs=2,
                                       name="ci")
                        nc.sync.dma_start(
                            out=ti[:, :cw], in_=src[r0:r0 + P, c0:c0 + cw])
                        so = cast.tile([P, 1024], bf16, tag="cso", name="cso")
                        for g in range(0, cw, 512):
                            gw = min(512, cw - g)
                            ps = psS.tile([P, KT], fp32, tag="psS", name="psS")
                            for j in range(gw // P):
                                nc.tensor.transpose(
                                    ps[:, j * P:(j + 1) * P],
                                    ti[:, g + j * P:g + (j + 1) * P],
                                    ident[:, :])
                            eng = cp_eng[cast_i[0] % 2]
                            cast_i[0] += 1
                            eng(so[:, g:g + gw], ps[:, :gw])
                        nc.gpsimd.dma_start(
                            out=dstT[c0:c0 + cw, r0:r0 + P].rearrange(
                                "(a p) r -> p a r", p=P),
                            in_=so[:, :cw].rearrange("p (a r) -> p a r", r=P))

            cast_T(w_qaT_d, w_qa, QLR, HID)
            cast_T(w_kvT_d, w_kv, KVD, HID)
            cast_T(w_qbT_d, w_qb, OC, QLR)
            cast_T(w_oT_d, w_o, HID, OC)
            for i in range(R // SBLK):
                cast_T(hbfT[i], hidden[i * SBLK:(i + 1) * SBLK, :], SBLK, HID)

            ropeT = singles.tile([P, s_len], fp32, name="ropeTs")
            nc.sync.dma_start(out=ropeT[:, :], in_=ropeT_in[:, :])

            def rope_apply(dst_hi, dst_lo, src, cols):
                w = src.shape[-1]
                a = rtmp.tile([32, P], fp32, tag="rt0", name="rt0")
                bb = rtmp.tile([32, P], fp32, tag="rt1", name="rt1")
                nc.vector.tensor_mul(a[:, :w], src[0:32, :], ropeT[0:32, cols])
                nc.vector.tensor_mul(bb[:, :w], src[32:64, :],
                                     ropeT[64:96, cols])
                nc.vector.tensor_sub(dst_hi, a[:, :w], bb[:, :w])
                nc.vector.tensor_mul(a[:, :w], src[32:64, :],
                                     ropeT[32:64, cols])
                nc.vector.tensor_mul(bb[:, :w], src[0:32, :],
                                     ropeT[96:128, cols])
                nc.vector.tensor_add(dst_lo, a[:, :w], bb[:, :w])

            # ---------------- main loop ----------------
            for b in range(b_count):
                kv_r = work.tile([P, NTOK, KVD], bf16, tag="kv_r",
                                 name="kv_r")
                keyT = work.tile([P, 5, s_len], bf16, tag="keyT", name="keyT")

                for blk in range(NB):
                    rows0 = b * s_len + blk * SBLK
                    nblk = b * NB + blk

                    hidT = work.tile([P, NKC, SBLK], bf16, tag="hidT",
                                     bufs=2, name="hidT")
                    nc.sync.dma_start(
                        out=hidT[:, :, :],
                        in_=hbfT[nblk].rearrange("(a p) s -> p a s", p=P))

                    # ---- q_a = hidden @ w_qa.T (row-major) ----
                    q_a_sb = work.tile([P, NSC, QLR], bf16, tag="q_a_sb",
                                       name="q_a_sb")
                    for ct in range(QLR // KT):
                        pss = [psA.tile([P, KT], fp32, tag="psA", name="psA")
                               for _ in range(NSC)]
                        for kc in range(NKC):
                            wt = strm.tile([P, KT], bf16, tag="w_qaT", bufs=4,
                                           name="w_qaT")
                            nc.sync.dma_start(
                                out=wt[:, :],
                                in_=w_qaT_d[kc * P:(kc + 1) * P,
                                            ct * KT:(ct + 1) * KT])
                            for sc in range(NSC):
                                nc.tensor.matmul(
                                    pss[sc][:, :],
                                    hidT[:, kc, sc * P:(sc + 1) * P],
                                    wt[:, :],
                                    start=(kc == 0), stop=(kc == NKC - 1))
                        for sc in range(NSC):
                            nc.scalar.copy(
                                out=q_a_sb[:, sc, ct * KT:(ct + 1) * KT],
                                in_=pss[sc][:, :])

                    # ---- kv = hidden @ w_kv.T (row-major) ----
                    for noff, nw in [(0, 512), (512, 128)]:
                        pss = [psA.tile([P, KT], fp32, tag="psA", name="psA")
                               for _ in range(NSC)]
                        for kc in range(NKC):
                            wt = strm.tile([P, KVD], bf16, tag="w_kvT",
                                           bufs=3, name="w_kvT")
                            nc.sync.dma_start(
                                out=wt[:, :],
                                in_=w_kvT_d[kc * P:(kc + 1) * P, :])
                            for sc in range(NSC):
                                nc.tensor.matmul(
                                    pss[sc][:, :nw],
                                    hidT[:, kc, sc * P:(sc + 1) * P],
                                    wt[:, noff:noff + nw],
                                    start=(kc == 0), stop=(kc == NKC - 1))
                        for sc in range(NSC):
                            nc.vector.tensor_copy(
                                out=kv_r[:, blk * NSC + sc, noff:noff + nw],
                                in_=pss[sc][:, :nw])

                    # ---- keyT assembly: nope via DMA transpose, rope chunk ----
                    for sc in range(NSC):
                        for j in range(4):
                            nc.sync.dma_start_transpose(
                                out=keyT[:, 1 + j,
                                         blk * SBLK + sc * P:
                                         blk * SBLK + (sc + 1) * P],
                                in_=kv_r[:, blk * NSC + sc,
                                         128 + j * P:128 + (j + 1) * P])
                    for sc in range(NSC):
                        tkc = blk * NSC + sc
                        cols = slice(blk * SBLK + sc * P,
                                     blk * SBLK + (sc + 1) * P)
                        t64 = rtmp.tile([P, 64], fp32, tag="t64", name="t64")
                        nc.vector.tensor_copy(out=t64[:, :],
                                              in_=kv_r[:, tkc, 0:64])
                        pt = psS.tile([P, KT], fp32, tag="psS", name="psS")
                        nc.tensor.transpose(pt[0:64, 0:P], t64[:, :],
                                            ident[:, :])
                        rope_apply(keyT[0:32, 0, cols], keyT[32:64, 0, cols],
                                   pt[0:64, 0:P], cols)

                    # ---- q_aT via DMA transpose ----
                    q_aT = work.tile([P, NQLC, SBLK], bf16, tag="q_aT",
                                     name="q_aT")
                    for sc in range(NSC):
                        for qlc in range(NQLC):
                            nc.sync.dma_start_transpose(
                                out=q_aT[:, qlc, sc * P:(sc + 1) * P],
                                in_=q_a_sb[:, sc, qlc * P:(qlc + 1) * P])

                    # ---- q = q_a @ w_qb.T (row-major) ----
                    q_sb = work.tile([P, NSC, OC], bf16, tag="q_sb",
                                     name="q_sb")
                    for ooff, otw in [(0, 512), (512, 512), (1024, 128)]:
                        pss = [psA.tile([P, KT], fp32, tag="psA", name="psA")
                               for _ in range(NSC)]
                        for qlc in range(NQLC):
                            wt = strm.tile([P, KT], bf16, tag="w_qbT", bufs=4,
                                           name="w_qbT")
                            nc.sync.dma_start(
                                out=wt[:, :otw],
                                in_=w_qbT_d[qlc * P:(qlc + 1) * P,
                                            ooff:ooff + otw])
                            for sc in range(NSC):
                                nc.tensor.matmul(
                                    pss[sc][:, :otw],
                                    q_aT[:, qlc, sc * P:(sc + 1) * P],
                                    wt[:, :otw],
                                    start=(qlc == 0), stop=(qlc == NQLC - 1))
                        for sc in range(NSC):
                            nc.scalar.copy(
                                out=q_sb[:, sc, ooff:ooff + otw],
                                in_=pss[sc][:, :otw])

                    # ---- queryT (d-major per head) + RoPE ----
                    queryT = work.tile([P, 2 * 5, SBLK], bf16, tag="queryT",
                                       name="queryT")
                    for hh in range(HPC):
                        for sc in range(NSC):
                            for slot, doff, dw in QK_CHUNKS[1:]:
                                so = hh * DHEAD + doff
                                nc.sync.dma_start_transpose(
                                    out=queryT[:, hh * 5 + slot,
                                               sc * P:(sc + 1) * P],
                                    in_=q_sb[:, sc, so:so + dw])
                        for sc in range(NSC):
                            lcols = slice(sc * P, (sc + 1) * P)
                            gcols = slice(blk * SBLK + sc * P,
                                          blk * SBLK + (sc + 1) * P)
                            t64 = rtmp.tile([P, 64], fp32, tag="t64",
                                            name="t64")
                            nc.vector.tensor_copy(
                                out=t64[:, :],
                                in_=q_sb[:, sc, hh * DHEAD:hh * DHEAD + 64])
                            pt = psS.tile([P, KT], fp32, tag="psS", name="psS")
                            nc.tensor.transpose(pt[0:64, 0:P], t64[:, :],
                                                ident[:, :])
                            rope_apply(queryT[0:32, hh * 5, lcols],
                                       queryT[32:64, hh * 5, lcols],
                                       pt[0:64, 0:P], gcols)

                    # ---- attention per head ----
                    attnoutT = work.tile([P, 9, SBLK], bf16, tag="attnoutT",
                                         name="attnoutT")
                    for hh in range(HPC):
                        PT = work.tile([P, NTOK, SBLK], bf16, tag="PT",
                                       name="PT")
                        for sc in range(NSC):
                            klen = (blk + 1) * KT
                            srow = work.tile([P, s_len], fp32, tag="srow",
                                             bufs=2, name="srow")
                            for kt in range(blk + 1):
                                ps_s = psS.tile([P, KT], fp32, tag="psS",
                                                name="psS")
                                for slot, doff, dw in QK_CHUNKS:
                                    nc.tensor.matmul(
                                        ps_s[:, :],
                                        queryT[0:dw, hh * 5 + slot,
                                               sc * P:(sc + 1) * P],
                                        keyT[0:dw, slot,
                                             kt * KT:(kt + 1) * KT],
                                        start=(slot == 0), stop=(slot == 4))
                                nc.vector.tensor_copy(
                                    out=srow[:, kt * KT:(kt + 1) * KT],
                                    in_=ps_s[:, :])
                            nc.gpsimd.affine_select(
                                out=srow[:, blk * KT:(blk + 1) * KT],
                                in_=srow[:, blk * KT:(blk + 1) * KT],
                                compare_op=GE, fill=NEG, base=sc * P,
                                pattern=[[-1, KT]], channel_multiplier=1)
                            mx = stats.tile([P, 1], fp32, tag="mx", name="mx")
                            nc.vector.tensor_reduce(
                                mx[:, :], srow[:, 0:klen], axis=AX, op=MAX)
                            negb = stats.tile([P, 1], fp32, tag="negb",
                                              name="negb")
                            nc.vector.tensor_scalar_mul(
                                negb[:, :], mx[:, :], -SCALE)
                            ssum = stats.tile([P, 1], fp32, tag="ssum",
                                              name="ssum")
                            nc.scalar.activation(
                                srow[:, 0:klen], srow[:, 0:klen], Exp,
                                bias=negb[:, :], scale=SCALE,
                                accum_out=ssum[:, :])
                            rec = stats.tile([P, 1], fp32, tag="rec",
                                             name="rec")
                            nc.vector.reciprocal(rec[:, :], ssum[:, :])
                            nc.vector.tensor_scalar_mul(
                                srow[:, 0:klen], srow[:, 0:klen], rec[:, :])
                            for g in range(blk + 1):
                                ps_t = psS.tile([P, KT], fp32, tag="psS",
                                                name="psS")
                                for j in range(4):
                                    nc.tensor.transpose(
                                        ps_t[:, j * P:(j + 1) * P],
                                        srow[:, (g * 4 + j) * P:
                                             (g * 4 + j + 1) * P],
                                        ident[:, :])
                                nc.vector.tensor_copy(
                                    out=PT[:, g * 4:(g + 1) * 4,
                                           sc * P:(sc + 1) * P],
                                    in_=ps_t.rearrange("p (a c) -> p a c",
                                                       a=4))
                        ntok_b = (blk + 1) * NSC
                        for ci, ro, vo, vw in AV_CHUNKS[hh]:
                            ps_v = psV.tile([P, SBLK], fp32, tag="psV",
                                            name="psV")
                            for tkc in range(ntok_b):
                                nc.tensor.matmul(
                                    ps_v[0:vw, :],
                                    kv_r[:, tkc, vo:vo + vw],
                                    PT[:, tkc, :],
                                    start=(tkc == 0), stop=(tkc == ntok_b - 1))
                            nc.scalar.copy(
                                out=attnoutT[ro:ro + vw, ci, :],
                                in_=ps_v[0:vw, :])

                    # ---- out_partial = attnout @ w_o.T ----
                    for ct in range(HID // KT):
                        wots = []
                        for hd in range(9):
                            wt = strm.tile([P, KT], bf16, tag="w_oT", bufs=11,
                                           name="w_oT")
                            nc.sync.dma_start(
                                out=wt[:, :],
                                in_=w_oT_d[hd * P:(hd + 1) * P,
                                           ct * KT:(ct + 1) * KT])
                            wots.append(wt)
                        for sc in range(NSC):
                            ps_o = psV.tile([P, SBLK], fp32, tag="psV",
                                            name="ps_o")
                            for hd in range(9):
                                nc.tensor.matmul(
                                    ps_o[:, :],
                                    attnoutT[:, hd, sc * P:(sc + 1) * P],
                                    wots[hd][:, :],
                                    start=(hd == 0), stop=(hd == 8))
                            ot = work.tile([P, KT], fp32, tag="out_sb",
                                           bufs=2, name="out_sb")
                            nc.vector.tensor_copy(out=ot[:, :], in_=ps_o[:, :])
                            nc.gpsimd.dma_start(
                                out=out_d[rows0 + sc * P:rows0 + (sc + 1) * P,
                                          ct * KT:(ct + 1) * KT],
                                in_=ot[:, :])

    nc.compile()
    return nc


def make_in_maps(inputs, b_count=B, s_len=S):
    hidden = np.ascontiguousarray(
        np.asarray(inputs["hidden_states"], dtype=np.float32).reshape(
            b_count * s_len, HID))
    cos = np.asarray(inputs["cos"], dtype=np.float32)
    sin = np.asarray(inputs["sin"], dtype=np.float32)
    ropeT = np.ascontiguousarray(
        np.concatenate([cos[0].T, sin[0].T], axis=0))  # [128, s_len]
    w_qa = np.ascontiguousarray(np.asarray(inputs["w_qa"], np.float32))
    w_qb = np.asarray(inputs["w_qb"], dtype=np.float32)
    w_kv = np.ascontiguousarray(np.asarray(inputs["w_kv"], np.float32))
    w_o = np.asarray(inputs["w_o"], dtype=np.float32)
    in_maps = []
    for c in range(N_CORES):
        in_maps.append({
            "hidden": hidden,
            "ropeT": ropeT,
            "w_qa": w_qa,
            "w_qb_h": np.ascontiguousarray(w_qb[c * OC:(c + 1) * OC, :]),
            "w_kv": w_kv,
            "w_o_h": np.ascontiguousarray(w_o[:, c * OC:(c + 1) * OC]),
        })
    return in_maps


_NC_CACHE = {}


def run_on_hw(inputs, trace=False):
    import os

    from concourse.bass_utils import run_bass_kernel_spmd

    if not trace:
        # axon client has no NTFF hook; a stray BASS_TRACE=1 would crash.
        os.environ["BASS_NEVER_TRACE"] = "1"

    key = "full"
    if key not in _NC_CACHE:
        _NC_CACHE[key] = build_nc()
    nc = _NC_CACHE[key]
    in_maps = make_in_maps(inputs)
    res = run_bass_kernel_spmd(nc, in_maps, core_ids=list(range(N_CORES)),
                               trace=trace)
    acc = np.zeros((B * S, HID), dtype=np.float32)
    for r in res.results:
        acc += r["out_part"]
    return acc.reshape(B, S, HID), res


def kernel(**inputs):
    out, _ = run_on_hw(inputs, trace=False)
    return out
